# revision 34
# baseline (speedup 1.0000x reference)
"""Edge-parallel GNN message-passing MLP on 8 TRN2 NeuronCores.

Computation (per edge e): out[e] = relu(concat(x[row[e]], edge_attr[e]) @ W1 + b1) @ W2 + b2

Strategy (v2 — no per-edge DMA descriptors):
  The baseline dma_gather spent ~16 us per 2048-edge tile generating DMA
  descriptors in GpSimd Q7 software (~8 ns/edge serial) — 800 us of the
  970 us kernel. This version eliminates the gather entirely:

  * x lives RESIDENT in SBUF in fp16, row-major chunks of 128 rows
    ([128 parts, 392 chunks, 64 feats], 50 KB/partition), loaded once.
  * Edges are globally sorted by row and dealt round-robin to the 8 cores,
    so every core has the same per-chunk edge-count profile (+-1). Each
    128-row chunk c gets a static slot count s_c = roundup4(ceil(total_c/8))
    baked into the program (compiled per data profile, cached).
  * Per chunk, a one-hot selection matrix S[r, s] (1 where slot s holds an
    edge whose row is chunk-local row r) is built ON THE HOST in fp8e4m3
    (exact for 0/1) and streamed in per 16-chunk supergroup — the PE
    accepts an fp16-stationary x fp8-moving matmul, so S costs 1 byte/elem
    of DMA and zero vector-engine time.
  * The "gather" is then a PE matmul: featsT_x = x_chunk.T @ S, done per
    chunk PAIR (pair-equal slot widths) into a bank-aligned [64, 2, 512]
    PSUM tile so each pair needs only ONE PSUM->SBUF drain.
    feats = [x-part; edge_attr-part] feature-major, then the 2-layer MLP
    runs on <=512-slot windows decoupled from chunks (W1/W2 stationary,
    one PSUM bank per window). The three drain kinds (expansion cast,
    relu+b1, +b2) are greedily load-balanced across the vector and scalar
    engines, which are the throughput bottleneck.
  * Everything is fp16 on the wire (x, ea, weights, h1, out) with fp32
    PSUM accumulation; rel err ~5e-4.
  * S / edge_attr loads and output stores are batched per supergroup of
    chunk pairs sized to 8 windows (~4 KB descriptors, full DMA rate).

Self-contained: shapes hardcoded for the 50000-node / 800000-edge problem.
"""

from contextlib import ExitStack

import numpy as np

import concourse.bacc as bacc_mod
import concourse.mybir as mybir
import concourse.tile as tile
from concourse.bass_utils import run_bass_kernel_spmd

N_CORES = 8
N_NODES = 50000
N_EDGES = 800000
F_IN = 64
HIDDEN = 128
F_OUT = 128

CHUNK_ROWS = 128
NCHUNK = 392                                       # chunks (incl. 1 pad chunk)
NPAD_ROWS = NCHUNK * CHUNK_ROWS                    # 50176
E_CORE = N_EDGES // N_CORES                        # 100000
SUPER = 16                                         # chunks per ea/out DMA batch
WIN = 512                                          # MLP window (1 PSUM bank)

F32 = mybir.dt.float32
FP16 = mybir.dt.float16
FP8 = mybir.dt.float8e4

RELU = mybir.ActivationFunctionType.Relu
IDENT = mybir.ActivationFunctionType.Identity
ADD_OP = mybir.AluOpType.add
MAX_OP = mybir.AluOpType.max


def _round_up(v, m):
    return (v + m - 1) // m * m


def build_program(s_prof: tuple[int, ...]):
    """Build the SPMD program for a per-chunk slot-count profile.

    s_prof must be quad-equal: s_prof[4i..4i+3] share one slot width so
    each 4-chunk group's expansion PSUM drains in one copy.
    """
    assert len(s_prof) == NCHUNK
    smax = max(s_prof)
    assert smax <= 512
    slot_base = np.concatenate([[0], np.cumsum(s_prof)]).astype(np.int64)
    total_slots = int(slot_base[-1])

    # supergroup boundaries: accumulate chunk pairs while width <= 8 windows
    groups = []
    g0 = 0
    while g0 < NCHUNK:
        g1 = g0
        width = 0
        while g1 < NCHUNK:
            pw = 4 * s_prof[g1]
            if width > 0 and width + pw > 8 * WIN:
                break
            width += pw
            g1 += 4
        groups.append((g0, g1, width))
        g0 = g1
    gwmax = max(w for _, _, w in groups)

    nc = bacc_mod.Bacc("TRN2")

    xr_d = nc.declare_dram_parameter("xr", [128, NCHUNK * F_IN], FP16, isOutput=False)
    s8_d = nc.declare_dram_parameter("s8", [128, max(total_slots, 1)], FP8, isOutput=False)
    ea_d = nc.declare_dram_parameter("eaT", [F_IN, max(total_slots, 1)], FP16, isOutput=False)
    w1_d = nc.declare_dram_parameter("w1", [2 * F_IN, HIDDEN], FP16, isOutput=False)
    w2_d = nc.declare_dram_parameter("w2", [HIDDEN, F_OUT], FP16, isOutput=False)
    b1_d = nc.declare_dram_parameter("b1c", [HIDDEN, 1], F32, isOutput=False)
    b2_d = nc.declare_dram_parameter("b2c", [F_OUT, 1], F32, isOutput=False)
    b2w_d = nc.declare_dram_parameter("b2w", [F_OUT, WIN], F32, isOutput=False)
    out_d = nc.declare_dram_parameter("outT", [F_OUT, max(total_slots, 1)], FP16, isOutput=True)

    with tile.TileContext(nc) as tc, ExitStack() as ctx:
        const = ctx.enter_context(tc.tile_pool(name="const", bufs=1))
        s8_p = ctx.enter_context(tc.tile_pool(name="s8", bufs=3))
        feats_p = ctx.enter_context(tc.tile_pool(name="feats", bufs=3))
        h1_p = ctx.enter_context(tc.tile_pool(name="h1", bufs=3))
        osb_p = ctx.enter_context(tc.tile_pool(name="osb", bufs=3))
        expps_p = ctx.enter_context(tc.tile_pool(name="expps", bufs=1, space="PSUM"))
        l1ps_p = ctx.enter_context(tc.tile_pool(name="l1ps", bufs=2, space="PSUM"))
        l2ps_p = ctx.enter_context(tc.tile_pool(name="l2ps", bufs=2, space="PSUM"))

        # ---- resident constants ----
        xr_t = const.tile([128, NCHUNK * F_IN], FP16, tag="xr")
        nc.sync.dma_start(out=xr_t, in_=xr_d[:, :])
        w1_t = const.tile([128, HIDDEN], FP16, tag="w1")
        nc.sync.dma_start(out=w1_t, in_=w1_d[:, :])
        w2_t = const.tile([128, F_OUT], FP16, tag="w2")
        nc.sync.dma_start(out=w2_t, in_=w2_d[:, :])
        b1_t = const.tile([128, 1], F32, tag="b1")
        nc.sync.dma_start(out=b1_t, in_=b1_d[:, :])
        b2_t = const.tile([128, 1], F32, tag="b2")
        nc.sync.dma_start(out=b2_t, in_=b2_d[:, :])
        # b2 replicated along free dim for the tensor_tensor drain
        b2w_t = const.tile([128, WIN], F32, tag="b2w")
        nc.sync.dma_start(out=b2w_t, in_=b2w_d[:, :])

        # greedy DVE/ACT load balancing for the three PSUM-drain op kinds
        eng_t = {"dve": 0.0, "act": 0.0}
        COST = {
            ("dve", "cast"): 600, ("act", "cast"): 620,
            ("dve", "relu"): 655, ("act", "relu"): 600,
            ("dve", "b2"): 640, ("act", "b2"): 590,
        }

        def pick(kind):
            d = eng_t["dve"] + COST[("dve", kind)]
            a = eng_t["act"] + COST[("act", kind)]
            eng = "dve" if d <= a else "act"
            eng_t[eng] += COST[(eng, kind)]
            return eng

        for g0, g1, gwidth in groups:
            gbase = int(slot_base[g0])
            feats = feats_p.tile([128, gwmax], FP16, tag="feats")
            osb = osb_p.tile([128, gwmax], FP16, tag="osb")
            s8_t = s8_p.tile([128, gwmax], FP8, tag="s8")

            # batched supergroup loads: one-hot S (fp8) + edge_attr (fp16)
            nc.sync.dma_start(
                out=s8_t[:, 0:gwidth], in_=s8_d[:, gbase : gbase + gwidth]
            )
            nc.sync.dma_start(
                out=feats[F_IN : 2 * F_IN, 0:gwidth],
                in_=ea_d[:, gbase : gbase + gwidth],
            )

            def emit_pair(c0):
                sp = s_prof[c0]
                off = int(slot_base[c0]) - gbase
                # quad PSUM: [64, 4, WIN] — each quarter bank-aligned
                eps = expps_p.tile([64, 4, WIN], F32, tag="eps", space="PSUM")
                for j in (0, 1, 2, 3):
                    c = c0 + j
                    nc.tensor.matmul(
                        out=eps[:, j, 0:sp],
                        lhsT=xr_t[:, c * F_IN : (c + 1) * F_IN],
                        rhs=s8_t[:, off + j * sp : off + (j + 1) * sp],
                        start=True,
                        stop=True,
                    )
                # one drain for the pair (either engine)
                dest = feats[0:F_IN, off : off + 4 * sp].rearrange(
                    "f (j s) -> f j s", j=4
                )
                if pick("cast") == "dve":
                    nc.vector.tensor_copy(out=dest, in_=eps[:, :, 0:sp])
                else:
                    nc.scalar.activation(
                        out=dest, in_=eps[:, :, 0:sp],
                        func=IDENT, bias=0.0, scale=1.0,
                    )

            def emit_window(w, wend):
                wl = wend - w
                l1 = l1ps_p.tile([128, WIN], F32, tag="l1", space="PSUM")
                nc.tensor.matmul(
                    out=l1[:, 0:wl],
                    lhsT=w1_t,
                    rhs=feats[:, w : w + wl],
                    start=True,
                    stop=True,
                )
                h1 = h1_p.tile([128, WIN], FP16, tag="h1")
                if pick("relu") == "act":
                    nc.scalar.activation(
                        out=h1[:, 0:wl],
                        in_=l1[:, 0:wl],
                        func=RELU,
                        bias=b1_t,
                        scale=1.0,
                    )
                else:
                    nc.vector.tensor_scalar(
                        out=h1[:, 0:wl],
                        in0=l1[:, 0:wl],
                        scalar1=b1_t,
                        scalar2=0.0,
                        op0=ADD_OP,
                        op1=MAX_OP,
                    )
                l2 = l2ps_p.tile([128, WIN], F32, tag="l2", space="PSUM")
                nc.tensor.matmul(
                    out=l2[:, 0:wl],
                    lhsT=w2_t,
                    rhs=h1[:, 0:wl],
                    start=True,
                    stop=True,
                )
                # drain+b2 (either engine)
                if pick("b2") == "dve":
                    nc.vector.tensor_tensor(
                        out=osb[:, w : w + wl],
                        in0=l2[:, 0:wl],
                        in1=b2w_t[:, 0:wl],
                        op=ADD_OP,
                    )
                else:
                    nc.scalar.activation(
                        out=osb[:, w : w + wl],
                        in_=l2[:, 0:wl],
                        func=IDENT,
                        bias=b2_t,
                        scale=1.0,
                    )

            # expansion pairs, then balanced MLP windows over the supergroup
            for c0 in range(g0, g1, 4):
                if s_prof[c0] > 0:
                    emit_pair(c0)
            nw = -(-gwidth // WIN)
            wsz = _round_up(-(-gwidth // nw), 4)
            bounds = [min(i * wsz, gwidth) for i in range(nw + 1)]
            for w, we in zip(bounds[:-1], bounds[1:]):
                if we > w:
                    emit_window(w, we)

            nc.sync.dma_start(
                out=out_d[:, gbase : gbase + gwidth], in_=osb[:, 0:gwidth]
            )

    nc.compile()
    return nc


_PROG_CACHE: dict[tuple, object] = {}


def _get_prog(s_prof: tuple[int, ...]):
    prog = _PROG_CACHE.get(s_prof)
    if prog is None:
        prog = build_program(s_prof)
        _PROG_CACHE[s_prof] = prog
    return prog


def _prepare(x, edge_index, edge_attr, W1, b1, W2, b2):
    x = np.ascontiguousarray(np.asarray(x, dtype=np.float32))
    row = np.ascontiguousarray(np.asarray(edge_index, dtype=np.int64)[0])
    ea = np.asarray(edge_attr, dtype=np.float32)
    w1 = np.asarray(W1, dtype=np.float32)
    w2 = np.asarray(W2, dtype=np.float32)
    b1v = np.asarray(b1, dtype=np.float32).reshape(HIDDEN, 1)
    b2v = np.asarray(b2, dtype=np.float32).reshape(F_OUT, 1)

    # global row-sort; deal sorted edges round-robin to cores
    order_g = np.argsort(row, kind="stable")
    t_c = np.bincount(row >> 7, minlength=NCHUNK)
    s_raw = [-(-int(t) // N_CORES) for t in t_c]
    # pair-equal slot widths (chunk pairs share one expansion-PSUM drain)
    s_prof = []
    for i in range(0, NCHUNK, 4):
        sp = _round_up(max(s_raw[i : i + 4]), 4)
        s_prof += [sp] * 4
    s_prof = tuple(s_prof)
    slot_base = np.concatenate([[0], np.cumsum(s_prof)]).astype(np.int64)
    total_slots = int(slot_base[-1])
    smax = max(s_prof)

    # x row-major chunks, fp16: xr[p, c*64+f] = x[128c+p, f]
    x_pad = np.zeros((NPAD_ROWS, F_IN), dtype=np.float16)
    x_pad[:N_NODES] = x.astype(np.float16)
    xr = np.ascontiguousarray(
        x_pad.reshape(NCHUNK, 128, F_IN).transpose(1, 0, 2)
    ).reshape(128, NCHUNK * F_IN)

    import ml_dtypes

    in_maps = []
    slot_maps = []  # per core: original-edge-id -> slot
    for k in range(N_CORES):
        gsel = order_g[k::N_CORES]  # original edge ids, row-sorted
        rk = row[gsel]
        ck = rk >> 7
        m = np.bincount(ck, minlength=NCHUNK)
        assert (m <= np.asarray(s_prof)).all()
        cum0 = np.concatenate([[0], np.cumsum(m)]).astype(np.int64)
        local = np.arange(len(gsel), dtype=np.int64) - cum0[ck]
        slots = slot_base[ck] + local

        # one-hot selection matrix: S[row - 128*chunk, slot] = 1
        s8 = np.zeros((128, total_slots), dtype=ml_dtypes.float8_e4m3fn)
        s8[rk & 127, slots] = 1.0

        eaT = np.zeros((F_IN, total_slots), dtype=np.float16)
        eaT[:, slots] = ea[gsel].astype(np.float16).T

        in_maps.append(
            {
                "xr": xr,
                "s8": s8,
                "eaT": eaT,
                "w1": w1.astype(np.float16),
                "w2": w2.astype(np.float16),
                "b1c": b1v,
                "b2c": b2v,
                "b2w": np.ascontiguousarray(
                    np.broadcast_to(b2v, (F_OUT, WIN)).astype(np.float32)
                ),
            }
        )
        slot_maps.append((gsel, slots))
    return s_prof, in_maps, slot_maps


def run_spmd(inputs: dict, trace: bool = False, **spmd_kwargs):
    """Run the kernel on all 8 cores. Returns (output, BassKernelResults)."""
    s_prof, in_maps, slot_maps = _prepare(
        inputs["x"], inputs["edge_index"], inputs["edge_attr"],
        inputs["W1"], inputs["b1"], inputs["W2"], inputs["b2"],
    )
    nc = _get_prog(s_prof)
    bres = run_bass_kernel_spmd(
        nc, in_maps, list(range(N_CORES)), trace=trace, **spmd_kwargs
    )
    out = np.empty((N_EDGES, F_OUT), dtype=np.float32)
    for k in range(N_CORES):
        gsel, slots = slot_maps[k]
        outT = bres.results[k]["outT"]  # [F_OUT, total_slots] fp16
        out[gsel] = outT[:, slots].T.astype(np.float32)
    return out, bres


def kernel(x, edge_index, edge_attr, u, batch, W1, b1, W2, b2):
    out, _ = run_spmd(
        {
            "x": x, "edge_index": edge_index, "edge_attr": edge_attr,
            "W1": W1, "b1": b1, "W2": W2, "b2": b2,
        }
    )
    return out
